# revision 32
# baseline (speedup 1.0000x reference)
"""ECE loss kernel for Trainium2 (Bass/Tile), data-parallel over 8 NeuronCores.

Math (per sample row of logits[N, C]):
  conf = max softmax(x) = max(E) / sum(E),  E = exp(x)
  acc  = (argmax(x) == label)  via  exp(g) == max(E), g = x[i, label_i]
  ece  = sum_b |conf_sum[b] - acc_sum[b]| / N   over 15 real bins

Per-core device work (125k rows as [125 partitions x 1000 samples x 100 cls]),
balanced across ALL engines (the previous version put everything on DVE):
  - DMA   (sync HWDGE only): 13 tiles, up to 5 MB each
  - ACT   : E = exp(x) in place; later all per-bin statistics via
            activation(Relu/Sign, bias=-C, accum_out=...) which gives a free
            per-partition sum of the activated values
  - DVE   : rowmax(E); rowsum for the small lead tiles; final rowsum over 25
            for the big tiles; recip/eq/mul/stt epilogue per chunk
  - GpSimd: two pairwise-ADD tree levels (100->50->25) in place on each big
            tile, via tensor_tensor(add) -- runs after DVE's rowmax read
            (Pool TT supports add/mult but not max)

Per-bin statistics (accumulated per chunk of samples so they overlap the
main loop instead of forming a serial tail):
  wt'(Cb) = sum relu(conf - Cb)        (ACT, 15 ops: Cb in {0} + C_0..C_13)
  nle_b   = sum (conf <= C_b)          (DVE tensor_scalar accum, 14 ops)
  q'(Tb)  = sum sign(v' - Tb)          (ACT, 15 ops) where
            v' = 2*sign(eg - maxE) - conf  (acc1: -conf; acc0: -2-conf),
            Tb = -C_b for b=0..13 and -1.0 for the total-acc count
Host recovers:
  T = wt'(0);  S_b = T - wt'_b - C_b*(N - nle_b)
  A_b = (N + q'_b)/2   (cumulative acc counts);  diffs give per-bin sums.
C_b is the exact f32 boundary: the largest f32 y with f32(15*y) <= b+1, so
binning matches the reference's ceil(conf*15) up to ~1-sample tie effects
(~1e-6 relative on the final ECE).
"""

import os

import numpy as np

import concourse.bass as bass
import concourse.mybir as mybir
import concourse.tile as tile
from concourse.bass_utils import run_bass_kernel_spmd

F32 = mybir.dt.float32
F16 = mybir.dt.float16
ALU = mybir.AluOpType
AX = mybir.AxisListType
ACTF = mybir.ActivationFunctionType

N = 1_000_000
C = 100
NCORES = 8
ROWS = N // NCORES          # 125000 rows per core
P = 125                     # SBUF partitions used
SPP = ROWS // P             # 1000 samples per partition

# small tiles at both ends: fast pipeline ramp-up AND a short serial tail
SIZES = [12, 13, 25, 50] + [100] * 8 + [50, 25, 13, 12]   # 16 tiles, 1000
HALF_TILE = 8              # after this tile, samples [0:HALF] are final
HALF = 600
DVE_FULL_K = 25            # tiles with k <= this do the row sum on DVE too

LAST_RESULTS = None         # stashed BassKernelResults for test harness


def _bin_thresholds():
    """C_b = largest f32 y such that f32(15*y) <= b+1, for b = 0..14."""
    thr = []
    for b in range(15):
        tgt = np.float32(b + 1)

        def f(v):
            return np.float32(np.float32(15.0) * v)

        y = np.float32((b + 1) / 15.0)
        if f(y) <= tgt:
            while True:
                y2 = np.nextafter(y, np.float32(np.inf))
                if f(y2) <= tgt:
                    y = y2
                else:
                    break
        else:
            while f(y) > tgt:
                y = np.nextafter(y, np.float32(-np.inf))
        thr.append(np.float32(y))
    return thr


THR = _bin_thresholds()                       # 15 values, b = 0..14

# bias constants shipped as a tiny input tensor (the const-AP pool only has
# 0.0/1.0 pre-registered):  [0] = 0.0 (wt base),  [1+b] = -C_b (wt relu),
# [15+b] = +C_b (av sign on v' = 2*sign(d) - conf),  [29] = +1.0 (acc count)
NCONST = 30
CVEC = np.zeros(NCONST, np.float32)
for _b in range(14):
    CVEC[1 + _b] = -THR[_b]
    CVEC[15 + _b] = THR[_b]
CVEC[29] = np.float32(1.0)


def _fix_sync(nc):
    """Instruction encodings only carry 2 sync-command slots (completion
    update takes one), so every instruction should hold <= 1 wait.  Tile's
    sem emission is not transitively minimal, so: (1) drop waits implied
    transitively through other waits / same-engine program order; (2) split
    any leftover multi-wait instruction into a chain of presync drains."""
    import bisect
    import re

    import bass_rust as _br

    TICK = re.compile(r"^(Activation|DVE|PE|Pool|SP|DMAHW\d+|DMASW\d+)_\d+$")
    ASYNC_T = {"InstDMACopy", "InstTriggerDma"}

    insts = []
    for bb in nc.m.functions[0].blocks:
        for ins in bb.instructions:
            insts.append(ins)
    n = len(insts)

    # producer map: tick sem -> sorted cumulative values + producing inst idx
    prod_vals, prod_idx = {}, {}
    own_updates = [[] for _ in range(n)]
    cum = {}
    for idx, ins in enumerate(insts):
        si = ins.sync_info
        if si is None:
            continue
        for u in si.on_update:
            nm = u.ant_name
            if not nm or not TICK.match(nm):
                continue
            if u.update_mode not in ("sem-inc", "sem-add-imm"):
                continue
            v = cum.get(nm, 0) + (u.update_value or 1)
            cum[nm] = v
            prod_vals.setdefault(nm, []).append(v)
            prod_idx.setdefault(nm, []).append(idx)
            own_updates[idx].append((nm, v))

    def producer(nm, val):
        vs = prod_vals.get(nm)
        if not vs:
            return None
        k = bisect.bisect_left(vs, val)
        if k >= len(vs):
            return None
        return prod_idx[nm][k]

    prev_idx = [None] * n
    last = {}
    for idx, ins in enumerate(insts):
        e = str(getattr(ins, "engine", None))
        prev_idx[idx] = last.get(e)
        last[e] = idx

    # before[i]: sem clock guaranteed when inst i issues (incl its waits)
    # after[i]: clock guaranteed when inst i COMPLETES (incl own updates)
    before = [None] * n
    after = [None] * n

    def wait_producers(i):
        si = insts[i].sync_info
        out = []
        for w in (si.on_wait if si else []):
            pi = None
            if w.ant_name and TICK.match(w.ant_name):
                pi = producer(w.ant_name, w.wait_value)
                if pi == i:
                    pi = None
            out.append((w, pi))
        return out

    def compute(idx):
        stack = [idx]
        while stack:
            i = stack[-1]
            if after[i] is not None:
                stack.pop()
                continue
            deps = []
            p = prev_idx[i]
            if p is not None and after[p] is None:
                deps.append(p)
            wps = wait_producers(i)
            for w, pi in wps:
                if pi is not None and after[pi] is None:
                    deps.append(pi)
            if deps:
                stack.extend(deps)
                continue
            stack.pop()
            c = {}
            if p is not None:
                src = before[p] if type(insts[p]).__name__ in ASYNC_T else after[p]
                for s, v in src.items():
                    if c.get(s, -1) < v:
                        c[s] = v
            for w, pi in wps:
                if pi is not None:
                    for s, v in after[pi].items():
                        if c.get(s, -1) < v:
                            c[s] = v
                if w.ant_name and TICK.match(w.ant_name):
                    if c.get(w.ant_name, -1) < w.wait_value:
                        c[w.ant_name] = w.wait_value
            before[i] = c
            a = dict(c)
            for nm, v in own_updates[i]:
                if a.get(nm, -1) < v:
                    a[nm] = v
            after[i] = a

    for i in range(n):
        compute(i)

    # pass 1: transitive reduction of each instruction's wait list
    for i, ins in enumerate(insts):
        si = ins.sync_info
        if si is None or len(si.on_wait) <= 1:
            continue
        if type(ins).__name__ == "InstEventSemaphore":
            continue
        waits = list(si.on_wait)
        p = prev_idx[i]
        base = {}
        if p is not None:
            src = before[p] if type(insts[p]).__name__ in ASYNC_T else after[p]
            base.update(src)
        closures = []
        for w in waits:
            cl = {}
            if w.ant_name and TICK.match(w.ant_name):
                pi = producer(w.ant_name, w.wait_value)
                if pi is not None and pi != i:
                    cl.update(after[pi])
                if cl.get(w.ant_name, -1) < w.wait_value:
                    cl[w.ant_name] = w.wait_value
            closures.append(cl)
        kept = []
        kept_cl = dict(base)
        for j, w in enumerate(waits):
            nm = w.ant_name
            if not (nm and TICK.match(nm)):
                kept.append(w)
                continue
            cov = dict(kept_cl)
            for j2 in range(j + 1, len(waits)):
                for s, v in closures[j2].items():
                    if cov.get(s, -1) < v:
                        cov[s] = v
            if cov.get(nm, -1) >= w.wait_value:
                continue
            kept.append(w)
            for s, v in closures[j].items():
                if kept_cl.get(s, -1) < v:
                    kept_cl[s] = v
        if len(kept) != len(waits):
            si.on_wait = kept
            ins.sync_info = si

    # pass 2: split any instruction still carrying > 1 wait into a chain of
    # same-engine presync drains (each drain fits a single sync command)
    for bb in nc.m.functions[0].blocks:
        while True:
            insns = list(bb.instructions)
            target = None
            for idx, ins in enumerate(insns):
                si = ins.sync_info
                if si is None:
                    continue
                if len(si.on_wait) > 1:
                    target = (idx, ins)
                    break
            if target is None:
                break
            idx, ins = target
            si = ins.sync_info
            waits = list(si.on_wait)
            if type(ins).__name__ == "InstDrain":
                room = max(0, 1 - len(si.on_update))
            else:
                room = 1
            keep, extra = waits[len(waits) - room:], waits[: len(waits) - room]
            pos = idx
            for i, w in enumerate(extra):
                nd = mybir.InstDrain(
                    name=f"{ins.name}-presync{i}", ins=[], outs=[],
                    bass_is_fusable=False,
                )
                nd.engine = ins.engine
                nd.sync_info = _br.SyncInfo(on_wait=[w], on_update=[])
                nc.register_instruction(nd, overwrite=True)
                bb.instructions.insert(pos, nd)
                pos += 1
            si.on_wait = keep
            ins.sync_info = si


def _build():
    nc = bass.Bass(trn_type="TRN2")
    x = nc.dram_tensor("x", [P, SPP * C], F16, kind="ExternalInput")
    g = nc.dram_tensor("g", [P, SPP], F16, kind="ExternalInput")
    mo = nc.dram_tensor("mo", [P, SPP], F32, kind="ExternalOutput")
    so = nc.dram_tensor("so", [P, SPP], F32, kind="ExternalOutput")
    ego = nc.dram_tensor("ego", [P, SPP], F16, kind="ExternalOutput")

    X = x[:, :].rearrange("p (k c) -> p k c", c=C)  # [125, 1000, 100]

    with tile.TileContext(nc) as tc:
        with (
            tc.tile_pool(name="xin", bufs=1) as xin,
            tc.tile_pool(name="persist", bufs=1) as persist,
        ):
            # first/second-half result buffers are SEPARATE tiles: Tile's
            # dependency tracking is tile-granular, so a half-output DMA
            # from a shared buffer would wait for ALL of its writers
            mA = persist.tile([P, HALF], F32)
            sA = persist.tile([P, HALF], F32)
            mB = persist.tile([P, SPP - HALF], F32)
            sB = persist.tile([P, SPP - HALF], F32)
            eg = persist.tile([P, SPP], F16)

            nc.scalar.dma_start(out=eg[:, :], in_=g[:, :])

            off = 0
            pending_sum = None     # deferred 25-wide row sum (prev tile)
            for t, k in enumerate(SIZES):
                off += k
                if off <= HALF:
                    m_c, s_c = mA, sA
                    sl = slice(off - k, off)
                else:
                    m_c, s_c = mB, sB
                    sl = slice(off - k - HALF, off - HALF)

                # explicit per-slot tags: Tile's free-pool reuse is LIFO,
                # which collapses the buffers and serializes the pipeline;
                # manual round-robin enforces reuse distance 8
                xt = xin.tile([P, 100, C], F16, tag=f"xt{t % 8}", name=f"xt{t}")
                nc.sync.dma_start(out=xt[:, :k, :], in_=X[:, off - k:off, :])
                nc.scalar.activation(xt[:, :k, :], xt[:, :k, :], ACTF.Exp)
                if t == 2:
                    nc.scalar.activation(eg[:, :], eg[:, :], ACTF.Exp)
                nc.vector.reduce_max(out=m_c[:, sl], in_=xt[:, :k, :], axis=AX.X)
                if k <= DVE_FULL_K:
                    nc.vector.reduce_sum(
                        out=s_c[:, sl], in_=xt[:, :k, :], axis=AX.X
                    )
                else:
                    # pairwise ADD tree on GpSimd (Pool TT supports add, not
                    # max), in place after DVE's rowmax read; the final
                    # 25-wide DVE row sum is DEFERRED one tile so DVE is not
                    # head-of-line blocked on the Pool engine
                    nc.gpsimd.tensor_tensor(
                        xt[:, :k, 0:50], xt[:, :k, 0:50], xt[:, :k, 50:100],
                        op=ALU.add,
                    )
                    nc.gpsimd.tensor_tensor(
                        xt[:, :k, 0:25], xt[:, :k, 0:25], xt[:, :k, 25:50],
                        op=ALU.add,
                    )
                    if pending_sum is not None:
                        pending_sum()
                    pending_sum = (
                        lambda xt=xt, k=k, s_c=s_c, sl=sl:
                        nc.vector.reduce_sum(
                            out=s_c[:, sl], in_=xt[:, :k, 0:25], axis=AX.X
                        )
                    )
                if t == HALF_TILE:
                    # first-half sums become final here (flush the deferred
                    # row sum); the output DMAs are emitted two tiles later
                    # so their waits never block the exp stream
                    if pending_sum is not None:
                        pending_sum()
                        pending_sum = None
                if t == HALF_TILE + 2:
                    nc.scalar.dma_start(out=mo[:, 0:HALF], in_=mA[:, :])
                    nc.scalar.dma_start(out=so[:, 0:HALF], in_=sA[:, :])
                    nc.scalar.dma_start(out=ego[:, :], in_=eg[:, :])

            if pending_sum is not None:
                pending_sum()
            nc.scalar.dma_start(out=mo[:, HALF:], in_=mB[:, :])
            nc.scalar.dma_start(out=so[:, HALF:], in_=sB[:, :])

    _fix_sync(nc)
    return nc


_NC_CACHE = {}


def _get_nc():
    if "nc" not in _NC_CACHE:
        _NC_CACHE["nc"] = _build()
    return _NC_CACHE["nc"]


def kernel(logits, labels):
    global LAST_RESULTS
    logits = np.asarray(logits)
    labels_i = np.asarray(labels).astype(np.int64)
    assert logits.shape == (N, C), logits.shape
    # fp16 halves the HBM traffic (the measured per-core DMA bandwidth with
    # all 8 cores active is ~130 GB/s, which is THE bottleneck); quantizing
    # the logits to fp16 moves the final ECE by only ~3e-4 relative
    logits = np.ascontiguousarray(logits.astype(np.float16))

    # host-side gather of the label logit (1% of input bytes); fp16 so the
    # device-side exp(g) matches the tile exp bit for bit
    gvals = logits[np.arange(N), labels_i]

    in_maps = []
    for c in range(NCORES):
        sl = slice(c * ROWS, (c + 1) * ROWS)
        in_maps.append(
            {
                "x": logits[sl].reshape(P, SPP * C),
                "g": gvals[sl].reshape(P, SPP),
            }
        )

    trace = bool(int(os.environ.get("ECE_TRACE", "0")))
    res = run_bass_kernel_spmd(
        _get_nc(), in_maps, core_ids=list(range(NCORES)), trace=trace
    )
    LAST_RESULTS = res

    # device returns per-sample (max E, sum E, exp(g)); the final 16-bin
    # histogram over 1M scalars is trivial host work
    m = np.concatenate([out["mo"].reshape(-1) for out in res.results])
    s = np.concatenate([out["so"].reshape(-1) for out in res.results])
    eg = np.concatenate([out["ego"].reshape(-1) for out in res.results])

    conf = m.astype(np.float64) / s.astype(np.float64)
    acc = (eg.astype(np.float32) == m).astype(np.float64)
    bin_ids = np.clip(np.ceil(conf * 15).astype(np.int64) - 1, 0, 15)
    cs = np.zeros(16)
    as_ = np.zeros(16)
    np.add.at(cs, bin_ids, conf)
    np.add.at(as_, bin_ids, acc)
    ece = np.abs(cs - as_).sum() / N
    return np.array([ece], dtype=np.float32)


# revision 33
# speedup vs baseline: 1.0053x; 1.0053x over previous
"""ECE loss kernel for Trainium2 (Bass/Tile), data-parallel over 8 NeuronCores.

Math (per sample row of logits[N, C]):
  conf = max softmax(x) = max(E) / sum(E),  E = exp(x)
  acc  = (argmax(x) == label)  via  exp(g) == max(E), g = x[i, label_i]
  ece  = sum_b |conf_sum[b] - acc_sum[b]| / N   over 15 real bins

Per-core device work (125k rows as [125 partitions x 1000 samples x 100 cls]),
balanced across ALL engines (the previous version put everything on DVE):
  - DMA   (sync HWDGE only): 13 tiles, up to 5 MB each
  - ACT   : E = exp(x) in place; later all per-bin statistics via
            activation(Relu/Sign, bias=-C, accum_out=...) which gives a free
            per-partition sum of the activated values
  - DVE   : rowmax(E); rowsum for the small lead tiles; final rowsum over 25
            for the big tiles; recip/eq/mul/stt epilogue per chunk
  - GpSimd: two pairwise-ADD tree levels (100->50->25) in place on each big
            tile, via tensor_tensor(add) -- runs after DVE's rowmax read
            (Pool TT supports add/mult but not max)

Per-bin statistics (accumulated per chunk of samples so they overlap the
main loop instead of forming a serial tail):
  wt'(Cb) = sum relu(conf - Cb)        (ACT, 15 ops: Cb in {0} + C_0..C_13)
  nle_b   = sum (conf <= C_b)          (DVE tensor_scalar accum, 14 ops)
  q'(Tb)  = sum sign(v' - Tb)          (ACT, 15 ops) where
            v' = 2*sign(eg - maxE) - conf  (acc1: -conf; acc0: -2-conf),
            Tb = -C_b for b=0..13 and -1.0 for the total-acc count
Host recovers:
  T = wt'(0);  S_b = T - wt'_b - C_b*(N - nle_b)
  A_b = (N + q'_b)/2   (cumulative acc counts);  diffs give per-bin sums.
C_b is the exact f32 boundary: the largest f32 y with f32(15*y) <= b+1, so
binning matches the reference's ceil(conf*15) up to ~1-sample tie effects
(~1e-6 relative on the final ECE).
"""

import os

import numpy as np

import concourse.bass as bass
import concourse.mybir as mybir
import concourse.tile as tile
from concourse.bass_utils import run_bass_kernel_spmd

F32 = mybir.dt.float32
F16 = mybir.dt.float16
ALU = mybir.AluOpType
AX = mybir.AxisListType
ACTF = mybir.ActivationFunctionType

N = 1_000_000
C = 100
NCORES = 8
ROWS = N // NCORES          # 125000 rows per core
P = 125                     # SBUF partitions used
SPP = ROWS // P             # 1000 samples per partition

# small tiles at both ends: fast pipeline ramp-up AND a short serial tail
SIZES = [12, 13, 25, 50] + [100] * 7 + [50, 50, 25, 25, 25, 13, 12]  # 18 tiles
HALF_TILE = 8              # after this tile, samples [0:HALF] are final
HALF = 600
DVE_FULL_K = 25            # tiles with k <= this do the row sum on DVE too

LAST_RESULTS = None         # stashed BassKernelResults for test harness


def _bin_thresholds():
    """C_b = largest f32 y such that f32(15*y) <= b+1, for b = 0..14."""
    thr = []
    for b in range(15):
        tgt = np.float32(b + 1)

        def f(v):
            return np.float32(np.float32(15.0) * v)

        y = np.float32((b + 1) / 15.0)
        if f(y) <= tgt:
            while True:
                y2 = np.nextafter(y, np.float32(np.inf))
                if f(y2) <= tgt:
                    y = y2
                else:
                    break
        else:
            while f(y) > tgt:
                y = np.nextafter(y, np.float32(-np.inf))
        thr.append(np.float32(y))
    return thr


THR = _bin_thresholds()                       # 15 values, b = 0..14

# bias constants shipped as a tiny input tensor (the const-AP pool only has
# 0.0/1.0 pre-registered):  [0] = 0.0 (wt base),  [1+b] = -C_b (wt relu),
# [15+b] = +C_b (av sign on v' = 2*sign(d) - conf),  [29] = +1.0 (acc count)
NCONST = 30
CVEC = np.zeros(NCONST, np.float32)
for _b in range(14):
    CVEC[1 + _b] = -THR[_b]
    CVEC[15 + _b] = THR[_b]
CVEC[29] = np.float32(1.0)


def _fix_sync(nc):
    """Instruction encodings only carry 2 sync-command slots (completion
    update takes one), so every instruction should hold <= 1 wait.  Tile's
    sem emission is not transitively minimal, so: (1) drop waits implied
    transitively through other waits / same-engine program order; (2) split
    any leftover multi-wait instruction into a chain of presync drains."""
    import bisect
    import re

    import bass_rust as _br

    TICK = re.compile(r"^(Activation|DVE|PE|Pool|SP|DMAHW\d+|DMASW\d+)_\d+$")
    ASYNC_T = {"InstDMACopy", "InstTriggerDma"}

    insts = []
    for bb in nc.m.functions[0].blocks:
        for ins in bb.instructions:
            insts.append(ins)
    n = len(insts)

    # producer map: tick sem -> sorted cumulative values + producing inst idx
    prod_vals, prod_idx = {}, {}
    own_updates = [[] for _ in range(n)]
    cum = {}
    for idx, ins in enumerate(insts):
        si = ins.sync_info
        if si is None:
            continue
        for u in si.on_update:
            nm = u.ant_name
            if not nm or not TICK.match(nm):
                continue
            if u.update_mode not in ("sem-inc", "sem-add-imm"):
                continue
            v = cum.get(nm, 0) + (u.update_value or 1)
            cum[nm] = v
            prod_vals.setdefault(nm, []).append(v)
            prod_idx.setdefault(nm, []).append(idx)
            own_updates[idx].append((nm, v))

    def producer(nm, val):
        vs = prod_vals.get(nm)
        if not vs:
            return None
        k = bisect.bisect_left(vs, val)
        if k >= len(vs):
            return None
        return prod_idx[nm][k]

    prev_idx = [None] * n
    last = {}
    for idx, ins in enumerate(insts):
        e = str(getattr(ins, "engine", None))
        prev_idx[idx] = last.get(e)
        last[e] = idx

    # before[i]: sem clock guaranteed when inst i issues (incl its waits)
    # after[i]: clock guaranteed when inst i COMPLETES (incl own updates)
    before = [None] * n
    after = [None] * n

    def wait_producers(i):
        si = insts[i].sync_info
        out = []
        for w in (si.on_wait if si else []):
            pi = None
            if w.ant_name and TICK.match(w.ant_name):
                pi = producer(w.ant_name, w.wait_value)
                if pi == i:
                    pi = None
            out.append((w, pi))
        return out

    def compute(idx):
        stack = [idx]
        while stack:
            i = stack[-1]
            if after[i] is not None:
                stack.pop()
                continue
            deps = []
            p = prev_idx[i]
            if p is not None and after[p] is None:
                deps.append(p)
            wps = wait_producers(i)
            for w, pi in wps:
                if pi is not None and after[pi] is None:
                    deps.append(pi)
            if deps:
                stack.extend(deps)
                continue
            stack.pop()
            c = {}
            if p is not None:
                src = before[p] if type(insts[p]).__name__ in ASYNC_T else after[p]
                for s, v in src.items():
                    if c.get(s, -1) < v:
                        c[s] = v
            for w, pi in wps:
                if pi is not None:
                    for s, v in after[pi].items():
                        if c.get(s, -1) < v:
                            c[s] = v
                if w.ant_name and TICK.match(w.ant_name):
                    if c.get(w.ant_name, -1) < w.wait_value:
                        c[w.ant_name] = w.wait_value
            before[i] = c
            a = dict(c)
            for nm, v in own_updates[i]:
                if a.get(nm, -1) < v:
                    a[nm] = v
            after[i] = a

    for i in range(n):
        compute(i)

    # pass 1: transitive reduction of each instruction's wait list
    for i, ins in enumerate(insts):
        si = ins.sync_info
        if si is None or len(si.on_wait) <= 1:
            continue
        if type(ins).__name__ == "InstEventSemaphore":
            continue
        waits = list(si.on_wait)
        p = prev_idx[i]
        base = {}
        if p is not None:
            src = before[p] if type(insts[p]).__name__ in ASYNC_T else after[p]
            base.update(src)
        closures = []
        for w in waits:
            cl = {}
            if w.ant_name and TICK.match(w.ant_name):
                pi = producer(w.ant_name, w.wait_value)
                if pi is not None and pi != i:
                    cl.update(after[pi])
                if cl.get(w.ant_name, -1) < w.wait_value:
                    cl[w.ant_name] = w.wait_value
            closures.append(cl)
        kept = []
        kept_cl = dict(base)
        for j, w in enumerate(waits):
            nm = w.ant_name
            if not (nm and TICK.match(nm)):
                kept.append(w)
                continue
            cov = dict(kept_cl)
            for j2 in range(j + 1, len(waits)):
                for s, v in closures[j2].items():
                    if cov.get(s, -1) < v:
                        cov[s] = v
            if cov.get(nm, -1) >= w.wait_value:
                continue
            kept.append(w)
            for s, v in closures[j].items():
                if kept_cl.get(s, -1) < v:
                    kept_cl[s] = v
        if len(kept) != len(waits):
            si.on_wait = kept
            ins.sync_info = si

    # pass 2: split any instruction still carrying > 1 wait into a chain of
    # same-engine presync drains (each drain fits a single sync command)
    for bb in nc.m.functions[0].blocks:
        while True:
            insns = list(bb.instructions)
            target = None
            for idx, ins in enumerate(insns):
                si = ins.sync_info
                if si is None:
                    continue
                if len(si.on_wait) > 1:
                    target = (idx, ins)
                    break
            if target is None:
                break
            idx, ins = target
            si = ins.sync_info
            waits = list(si.on_wait)
            if type(ins).__name__ == "InstDrain":
                room = max(0, 1 - len(si.on_update))
            else:
                room = 1
            keep, extra = waits[len(waits) - room:], waits[: len(waits) - room]
            pos = idx
            for i, w in enumerate(extra):
                nd = mybir.InstDrain(
                    name=f"{ins.name}-presync{i}", ins=[], outs=[],
                    bass_is_fusable=False,
                )
                nd.engine = ins.engine
                nd.sync_info = _br.SyncInfo(on_wait=[w], on_update=[])
                nc.register_instruction(nd, overwrite=True)
                bb.instructions.insert(pos, nd)
                pos += 1
            si.on_wait = keep
            ins.sync_info = si


def _build():
    nc = bass.Bass(trn_type="TRN2")
    x = nc.dram_tensor("x", [P, SPP * C], F16, kind="ExternalInput")
    g = nc.dram_tensor("g", [P, SPP], F16, kind="ExternalInput")
    mo = nc.dram_tensor("mo", [P, SPP], F32, kind="ExternalOutput")
    so = nc.dram_tensor("so", [P, SPP], F32, kind="ExternalOutput")
    ego = nc.dram_tensor("ego", [P, SPP], F16, kind="ExternalOutput")

    X = x[:, :].rearrange("p (k c) -> p k c", c=C)  # [125, 1000, 100]

    with tile.TileContext(nc) as tc:
        with (
            tc.tile_pool(name="xin", bufs=1) as xin,
            tc.tile_pool(name="persist", bufs=1) as persist,
        ):
            # first/second-half result buffers are SEPARATE tiles: Tile's
            # dependency tracking is tile-granular, so a half-output DMA
            # from a shared buffer would wait for ALL of its writers
            mA = persist.tile([P, HALF], F32)
            sA = persist.tile([P, HALF], F32)
            mB = persist.tile([P, SPP - HALF], F32)
            sB = persist.tile([P, SPP - HALF], F32)
            eg = persist.tile([P, SPP], F16)

            nc.scalar.dma_start(out=eg[:, :], in_=g[:, :])

            off = 0
            pending_sum = None     # deferred 25-wide row sum (prev tile)
            for t, k in enumerate(SIZES):
                off += k
                if off <= HALF:
                    m_c, s_c = mA, sA
                    sl = slice(off - k, off)
                else:
                    m_c, s_c = mB, sB
                    sl = slice(off - k - HALF, off - HALF)

                # explicit per-slot tags: Tile's free-pool reuse is LIFO,
                # which collapses the buffers and serializes the pipeline;
                # manual round-robin enforces reuse distance 8
                xt = xin.tile([P, 100, C], F16, tag=f"xt{t % 8}", name=f"xt{t}")
                nc.sync.dma_start(out=xt[:, :k, :], in_=X[:, off - k:off, :])
                nc.scalar.activation(xt[:, :k, :], xt[:, :k, :], ACTF.Exp)
                if t == 2:
                    nc.scalar.activation(eg[:, :], eg[:, :], ACTF.Exp)
                nc.vector.reduce_max(out=m_c[:, sl], in_=xt[:, :k, :], axis=AX.X)
                if k <= DVE_FULL_K:
                    nc.vector.reduce_sum(
                        out=s_c[:, sl], in_=xt[:, :k, :], axis=AX.X
                    )
                else:
                    # pairwise ADD tree on GpSimd (Pool TT supports add, not
                    # max), in place after DVE's rowmax read; the final
                    # 25-wide DVE row sum is DEFERRED one tile so DVE is not
                    # head-of-line blocked on the Pool engine
                    nc.gpsimd.tensor_tensor(
                        xt[:, :k, 0:50], xt[:, :k, 0:50], xt[:, :k, 50:100],
                        op=ALU.add,
                    )
                    nc.gpsimd.tensor_tensor(
                        xt[:, :k, 0:25], xt[:, :k, 0:25], xt[:, :k, 25:50],
                        op=ALU.add,
                    )
                    if pending_sum is not None:
                        pending_sum()
                    pending_sum = (
                        lambda xt=xt, k=k, s_c=s_c, sl=sl:
                        nc.vector.reduce_sum(
                            out=s_c[:, sl], in_=xt[:, :k, 0:25], axis=AX.X
                        )
                    )
                if t == HALF_TILE:
                    # first-half sums become final here (flush the deferred
                    # row sum); the output DMAs are emitted one tile later
                    # so their waits never block the exp stream
                    if pending_sum is not None:
                        pending_sum()
                        pending_sum = None
                if t == HALF_TILE + 1:
                    nc.scalar.dma_start(out=mo[:, 0:HALF], in_=mA[:, :])
                    nc.scalar.dma_start(out=so[:, 0:HALF], in_=sA[:, :])
                    nc.scalar.dma_start(out=ego[:, :], in_=eg[:, :])

            if pending_sum is not None:
                pending_sum()
            nc.scalar.dma_start(out=mo[:, HALF:], in_=mB[:, :])
            nc.scalar.dma_start(out=so[:, HALF:], in_=sB[:, :])

    _fix_sync(nc)
    return nc


_NC_CACHE = {}


def _get_nc():
    if "nc" not in _NC_CACHE:
        _NC_CACHE["nc"] = _build()
    return _NC_CACHE["nc"]


def kernel(logits, labels):
    global LAST_RESULTS
    logits = np.asarray(logits)
    labels_i = np.asarray(labels).astype(np.int64)
    assert logits.shape == (N, C), logits.shape
    # fp16 halves the HBM traffic (the measured per-core DMA bandwidth with
    # all 8 cores active is ~130 GB/s, which is THE bottleneck); quantizing
    # the logits to fp16 moves the final ECE by only ~3e-4 relative
    logits = np.ascontiguousarray(logits.astype(np.float16))

    # host-side gather of the label logit (1% of input bytes); fp16 so the
    # device-side exp(g) matches the tile exp bit for bit
    gvals = logits[np.arange(N), labels_i]

    in_maps = []
    for c in range(NCORES):
        sl = slice(c * ROWS, (c + 1) * ROWS)
        in_maps.append(
            {
                "x": logits[sl].reshape(P, SPP * C),
                "g": gvals[sl].reshape(P, SPP),
            }
        )

    trace = bool(int(os.environ.get("ECE_TRACE", "0")))
    res = run_bass_kernel_spmd(
        _get_nc(), in_maps, core_ids=list(range(NCORES)), trace=trace
    )
    LAST_RESULTS = res

    # device returns per-sample (max E, sum E, exp(g)); the final 16-bin
    # histogram over 1M scalars is trivial host work
    m = np.concatenate([out["mo"].reshape(-1) for out in res.results])
    s = np.concatenate([out["so"].reshape(-1) for out in res.results])
    eg = np.concatenate([out["ego"].reshape(-1) for out in res.results])

    conf = m.astype(np.float64) / s.astype(np.float64)
    acc = (eg.astype(np.float32) == m).astype(np.float64)
    bin_ids = np.clip(np.ceil(conf * 15).astype(np.int64) - 1, 0, 15)
    cs = np.zeros(16)
    as_ = np.zeros(16)
    np.add.at(cs, bin_ids, conf)
    np.add.at(as_, bin_ids, acc)
    ece = np.abs(cs - as_).sum() / N
    return np.array([ece], dtype=np.float32)


# revision 34
# speedup vs baseline: 1.0253x; 1.0199x over previous
"""ECE loss kernel for Trainium2 (Bass/Tile), data-parallel over 8 NeuronCores.

Math (per sample row of logits[N, C]):
  conf = max softmax(x) = max(E) / sum(E),  E = exp(x)
  acc  = (argmax(x) == label)  via  exp(g) == max(E), g = x[i, label_i]
  ece  = sum_b |conf_sum[b] - acc_sum[b]| / N   over 15 real bins

Per-core device work (125k rows as [125 partitions x 1000 samples x 100 cls]),
balanced across ALL engines (the previous version put everything on DVE):
  - DMA   (sync HWDGE only): 13 tiles, up to 5 MB each
  - ACT   : E = exp(x) in place; later all per-bin statistics via
            activation(Relu/Sign, bias=-C, accum_out=...) which gives a free
            per-partition sum of the activated values
  - DVE   : rowmax(E); rowsum for the small lead tiles; final rowsum over 25
            for the big tiles; recip/eq/mul/stt epilogue per chunk
  - GpSimd: two pairwise-ADD tree levels (100->50->25) in place on each big
            tile, via tensor_tensor(add) -- runs after DVE's rowmax read
            (Pool TT supports add/mult but not max)

Per-bin statistics (accumulated per chunk of samples so they overlap the
main loop instead of forming a serial tail):
  wt'(Cb) = sum relu(conf - Cb)        (ACT, 15 ops: Cb in {0} + C_0..C_13)
  nle_b   = sum (conf <= C_b)          (DVE tensor_scalar accum, 14 ops)
  q'(Tb)  = sum sign(v' - Tb)          (ACT, 15 ops) where
            v' = 2*sign(eg - maxE) - conf  (acc1: -conf; acc0: -2-conf),
            Tb = -C_b for b=0..13 and -1.0 for the total-acc count
Host recovers:
  T = wt'(0);  S_b = T - wt'_b - C_b*(N - nle_b)
  A_b = (N + q'_b)/2   (cumulative acc counts);  diffs give per-bin sums.
C_b is the exact f32 boundary: the largest f32 y with f32(15*y) <= b+1, so
binning matches the reference's ceil(conf*15) up to ~1-sample tie effects
(~1e-6 relative on the final ECE).
"""

import os

import numpy as np

import concourse.bass as bass
import concourse.mybir as mybir
import concourse.tile as tile
from concourse.bass_utils import run_bass_kernel_spmd

F32 = mybir.dt.float32
F16 = mybir.dt.float16
ALU = mybir.AluOpType
AX = mybir.AxisListType
ACTF = mybir.ActivationFunctionType

N = 1_000_000
C = 100
NCORES = 8
ROWS = N // NCORES          # 125000 rows per core
P = 125                     # SBUF partitions used
SPP = ROWS // P             # 1000 samples per partition

# small tiles at both ends: fast pipeline ramp-up AND a short serial tail
SIZES = [12, 13, 25, 50] + [100] * 6 + [50] * 4 + [25] * 4  # 18 tiles, 1000
HALF_TILE = 8              # after this tile, samples [0:HALF] are final
HALF = 600
DVE_FULL_K = 25            # tiles with k <= this do the row sum on DVE too

LAST_RESULTS = None         # stashed BassKernelResults for test harness


def _bin_thresholds():
    """C_b = largest f32 y such that f32(15*y) <= b+1, for b = 0..14."""
    thr = []
    for b in range(15):
        tgt = np.float32(b + 1)

        def f(v):
            return np.float32(np.float32(15.0) * v)

        y = np.float32((b + 1) / 15.0)
        if f(y) <= tgt:
            while True:
                y2 = np.nextafter(y, np.float32(np.inf))
                if f(y2) <= tgt:
                    y = y2
                else:
                    break
        else:
            while f(y) > tgt:
                y = np.nextafter(y, np.float32(-np.inf))
        thr.append(np.float32(y))
    return thr


THR = _bin_thresholds()                       # 15 values, b = 0..14

# bias constants shipped as a tiny input tensor (the const-AP pool only has
# 0.0/1.0 pre-registered):  [0] = 0.0 (wt base),  [1+b] = -C_b (wt relu),
# [15+b] = +C_b (av sign on v' = 2*sign(d) - conf),  [29] = +1.0 (acc count)
NCONST = 30
CVEC = np.zeros(NCONST, np.float32)
for _b in range(14):
    CVEC[1 + _b] = -THR[_b]
    CVEC[15 + _b] = THR[_b]
CVEC[29] = np.float32(1.0)


def _fix_sync(nc):
    """Instruction encodings only carry 2 sync-command slots (completion
    update takes one), so every instruction should hold <= 1 wait.  Tile's
    sem emission is not transitively minimal, so: (1) drop waits implied
    transitively through other waits / same-engine program order; (2) split
    any leftover multi-wait instruction into a chain of presync drains."""
    import bisect
    import re

    import bass_rust as _br

    TICK = re.compile(r"^(Activation|DVE|PE|Pool|SP|DMAHW\d+|DMASW\d+)_\d+$")
    ASYNC_T = {"InstDMACopy", "InstTriggerDma"}

    insts = []
    for bb in nc.m.functions[0].blocks:
        for ins in bb.instructions:
            insts.append(ins)
    n = len(insts)

    # producer map: tick sem -> sorted cumulative values + producing inst idx
    prod_vals, prod_idx = {}, {}
    own_updates = [[] for _ in range(n)]
    cum = {}
    for idx, ins in enumerate(insts):
        si = ins.sync_info
        if si is None:
            continue
        for u in si.on_update:
            nm = u.ant_name
            if not nm or not TICK.match(nm):
                continue
            if u.update_mode not in ("sem-inc", "sem-add-imm"):
                continue
            v = cum.get(nm, 0) + (u.update_value or 1)
            cum[nm] = v
            prod_vals.setdefault(nm, []).append(v)
            prod_idx.setdefault(nm, []).append(idx)
            own_updates[idx].append((nm, v))

    def producer(nm, val):
        vs = prod_vals.get(nm)
        if not vs:
            return None
        k = bisect.bisect_left(vs, val)
        if k >= len(vs):
            return None
        return prod_idx[nm][k]

    prev_idx = [None] * n
    last = {}
    for idx, ins in enumerate(insts):
        e = str(getattr(ins, "engine", None))
        prev_idx[idx] = last.get(e)
        last[e] = idx

    # before[i]: sem clock guaranteed when inst i issues (incl its waits)
    # after[i]: clock guaranteed when inst i COMPLETES (incl own updates)
    before = [None] * n
    after = [None] * n

    def wait_producers(i):
        si = insts[i].sync_info
        out = []
        for w in (si.on_wait if si else []):
            pi = None
            if w.ant_name and TICK.match(w.ant_name):
                pi = producer(w.ant_name, w.wait_value)
                if pi == i:
                    pi = None
            out.append((w, pi))
        return out

    def compute(idx):
        stack = [idx]
        while stack:
            i = stack[-1]
            if after[i] is not None:
                stack.pop()
                continue
            deps = []
            p = prev_idx[i]
            if p is not None and after[p] is None:
                deps.append(p)
            wps = wait_producers(i)
            for w, pi in wps:
                if pi is not None and after[pi] is None:
                    deps.append(pi)
            if deps:
                stack.extend(deps)
                continue
            stack.pop()
            c = {}
            if p is not None:
                src = before[p] if type(insts[p]).__name__ in ASYNC_T else after[p]
                for s, v in src.items():
                    if c.get(s, -1) < v:
                        c[s] = v
            for w, pi in wps:
                if pi is not None:
                    for s, v in after[pi].items():
                        if c.get(s, -1) < v:
                            c[s] = v
                if w.ant_name and TICK.match(w.ant_name):
                    if c.get(w.ant_name, -1) < w.wait_value:
                        c[w.ant_name] = w.wait_value
            before[i] = c
            a = dict(c)
            for nm, v in own_updates[i]:
                if a.get(nm, -1) < v:
                    a[nm] = v
            after[i] = a

    for i in range(n):
        compute(i)

    # pass 1: transitive reduction of each instruction's wait list
    for i, ins in enumerate(insts):
        si = ins.sync_info
        if si is None or len(si.on_wait) <= 1:
            continue
        if type(ins).__name__ == "InstEventSemaphore":
            continue
        waits = list(si.on_wait)
        p = prev_idx[i]
        base = {}
        if p is not None:
            src = before[p] if type(insts[p]).__name__ in ASYNC_T else after[p]
            base.update(src)
        closures = []
        for w in waits:
            cl = {}
            if w.ant_name and TICK.match(w.ant_name):
                pi = producer(w.ant_name, w.wait_value)
                if pi is not None and pi != i:
                    cl.update(after[pi])
                if cl.get(w.ant_name, -1) < w.wait_value:
                    cl[w.ant_name] = w.wait_value
            closures.append(cl)
        kept = []
        kept_cl = dict(base)
        for j, w in enumerate(waits):
            nm = w.ant_name
            if not (nm and TICK.match(nm)):
                kept.append(w)
                continue
            cov = dict(kept_cl)
            for j2 in range(j + 1, len(waits)):
                for s, v in closures[j2].items():
                    if cov.get(s, -1) < v:
                        cov[s] = v
            if cov.get(nm, -1) >= w.wait_value:
                continue
            kept.append(w)
            for s, v in closures[j].items():
                if kept_cl.get(s, -1) < v:
                    kept_cl[s] = v
        if len(kept) != len(waits):
            si.on_wait = kept
            ins.sync_info = si

    # pass 2: split any instruction still carrying > 1 wait into a chain of
    # same-engine presync drains (each drain fits a single sync command)
    for bb in nc.m.functions[0].blocks:
        while True:
            insns = list(bb.instructions)
            target = None
            for idx, ins in enumerate(insns):
                si = ins.sync_info
                if si is None:
                    continue
                if len(si.on_wait) > 1:
                    target = (idx, ins)
                    break
            if target is None:
                break
            idx, ins = target
            si = ins.sync_info
            waits = list(si.on_wait)
            if type(ins).__name__ == "InstDrain":
                room = max(0, 1 - len(si.on_update))
            else:
                room = 1
            keep, extra = waits[len(waits) - room:], waits[: len(waits) - room]
            pos = idx
            for i, w in enumerate(extra):
                nd = mybir.InstDrain(
                    name=f"{ins.name}-presync{i}", ins=[], outs=[],
                    bass_is_fusable=False,
                )
                nd.engine = ins.engine
                nd.sync_info = _br.SyncInfo(on_wait=[w], on_update=[])
                nc.register_instruction(nd, overwrite=True)
                bb.instructions.insert(pos, nd)
                pos += 1
            si.on_wait = keep
            ins.sync_info = si


def _build():
    nc = bass.Bass(trn_type="TRN2")
    x = nc.dram_tensor("x", [P, SPP * C], F16, kind="ExternalInput")
    g = nc.dram_tensor("g", [P, SPP], F16, kind="ExternalInput")
    mo = nc.dram_tensor("mo", [P, SPP], F32, kind="ExternalOutput")
    so = nc.dram_tensor("so", [P, SPP], F32, kind="ExternalOutput")
    ego = nc.dram_tensor("ego", [P, SPP], F16, kind="ExternalOutput")

    X = x[:, :].rearrange("p (k c) -> p k c", c=C)  # [125, 1000, 100]

    with tile.TileContext(nc) as tc:
        with (
            tc.tile_pool(name="xin", bufs=1) as xin,
            tc.tile_pool(name="persist", bufs=1) as persist,
        ):
            # first/second-half result buffers are SEPARATE tiles: Tile's
            # dependency tracking is tile-granular, so a half-output DMA
            # from a shared buffer would wait for ALL of its writers
            mA = persist.tile([P, HALF], F32)
            sA = persist.tile([P, HALF], F32)
            mB = persist.tile([P, SPP - HALF], F32)
            sB = persist.tile([P, SPP - HALF], F32)
            eg = persist.tile([P, SPP], F16)

            nc.scalar.dma_start(out=eg[:, :], in_=g[:, :])

            off = 0
            pending_sum = None     # deferred 25-wide row sum (prev tile)
            for t, k in enumerate(SIZES):
                off += k
                if off <= HALF:
                    m_c, s_c = mA, sA
                    sl = slice(off - k, off)
                else:
                    m_c, s_c = mB, sB
                    sl = slice(off - k - HALF, off - HALF)

                # explicit per-slot tags: Tile's free-pool reuse is LIFO,
                # which collapses the buffers and serializes the pipeline;
                # manual round-robin enforces reuse distance 8
                xt = xin.tile([P, 100, C], F16, tag=f"xt{t % 8}", name=f"xt{t}")
                nc.sync.dma_start(out=xt[:, :k, :], in_=X[:, off - k:off, :])
                nc.scalar.activation(xt[:, :k, :], xt[:, :k, :], ACTF.Exp)
                if t == 2:
                    nc.scalar.activation(eg[:, :], eg[:, :], ACTF.Exp)
                nc.vector.reduce_max(out=m_c[:, sl], in_=xt[:, :k, :], axis=AX.X)
                if k <= DVE_FULL_K:
                    nc.vector.reduce_sum(
                        out=s_c[:, sl], in_=xt[:, :k, :], axis=AX.X
                    )
                else:
                    # pairwise ADD tree on GpSimd (Pool TT supports add, not
                    # max), in place after DVE's rowmax read; the final
                    # 25-wide DVE row sum is DEFERRED one tile so DVE is not
                    # head-of-line blocked on the Pool engine
                    nc.gpsimd.tensor_tensor(
                        xt[:, :k, 0:50], xt[:, :k, 0:50], xt[:, :k, 50:100],
                        op=ALU.add,
                    )
                    nc.gpsimd.tensor_tensor(
                        xt[:, :k, 0:25], xt[:, :k, 0:25], xt[:, :k, 25:50],
                        op=ALU.add,
                    )
                    if pending_sum is not None:
                        pending_sum()
                    pending_sum = (
                        lambda xt=xt, k=k, s_c=s_c, sl=sl:
                        nc.vector.reduce_sum(
                            out=s_c[:, sl], in_=xt[:, :k, 0:25], axis=AX.X
                        )
                    )
                if t == HALF_TILE:
                    # first-half sums become final here (flush the deferred
                    # row sum); the output DMAs are emitted one tile later
                    # so their waits never block the exp stream
                    if pending_sum is not None:
                        pending_sum()
                        pending_sum = None
                if t == HALF_TILE + 1:
                    nc.scalar.dma_start(out=mo[:, 0:HALF], in_=mA[:, :])
                    nc.scalar.dma_start(out=so[:, 0:HALF], in_=sA[:, :])
                    nc.scalar.dma_start(out=ego[:, :], in_=eg[:, :])

            if pending_sum is not None:
                pending_sum()
            nc.scalar.dma_start(out=mo[:, HALF:], in_=mB[:, :])
            nc.scalar.dma_start(out=so[:, HALF:], in_=sB[:, :])

    _fix_sync(nc)
    return nc


_NC_CACHE = {}


def _get_nc():
    if "nc" not in _NC_CACHE:
        _NC_CACHE["nc"] = _build()
    return _NC_CACHE["nc"]


def kernel(logits, labels):
    global LAST_RESULTS
    logits = np.asarray(logits)
    labels_i = np.asarray(labels).astype(np.int64)
    assert logits.shape == (N, C), logits.shape
    # fp16 halves the HBM traffic (the measured per-core DMA bandwidth with
    # all 8 cores active is ~130 GB/s, which is THE bottleneck); quantizing
    # the logits to fp16 moves the final ECE by only ~3e-4 relative
    logits = np.ascontiguousarray(logits.astype(np.float16))

    # host-side gather of the label logit (1% of input bytes); fp16 so the
    # device-side exp(g) matches the tile exp bit for bit
    gvals = logits[np.arange(N), labels_i]

    in_maps = []
    for c in range(NCORES):
        sl = slice(c * ROWS, (c + 1) * ROWS)
        in_maps.append(
            {
                "x": logits[sl].reshape(P, SPP * C),
                "g": gvals[sl].reshape(P, SPP),
            }
        )

    trace = bool(int(os.environ.get("ECE_TRACE", "0")))
    res = run_bass_kernel_spmd(
        _get_nc(), in_maps, core_ids=list(range(NCORES)), trace=trace
    )
    LAST_RESULTS = res

    # device returns per-sample (max E, sum E, exp(g)); the final 16-bin
    # histogram over 1M scalars is trivial host work
    m = np.concatenate([out["mo"].reshape(-1) for out in res.results])
    s = np.concatenate([out["so"].reshape(-1) for out in res.results])
    eg = np.concatenate([out["ego"].reshape(-1) for out in res.results])

    conf = m.astype(np.float64) / s.astype(np.float64)
    acc = (eg.astype(np.float32) == m).astype(np.float64)
    bin_ids = np.clip(np.ceil(conf * 15).astype(np.int64) - 1, 0, 15)
    cs = np.zeros(16)
    as_ = np.zeros(16)
    np.add.at(cs, bin_ids, conf)
    np.add.at(as_, bin_ids, acc)
    ece = np.abs(cs - as_).sum() / N
    return np.array([ece], dtype=np.float32)
